# revision 26
# baseline (speedup 1.0000x reference)
"""MoE (8 routed experts, top-2, + shared expert) on 8 TRN2 NeuronCores.

Strategy: expert-parallel with load-balanced segmentation. Host computes
the gate (fp32 numpy, mirroring the reference), then packs the 8192
routed (token, expert) pairs into 16 expert-pure segments — 8 of size S1
and 8 of size S2 (sizes chosen per the actual expert counts so
S1+S2 ~= 1058 vs the 1152 max-count padding of naive expert-parallel).
Each core runs three SwiGLU jobs: a 512-token slice of the shared
expert (first: smallest x, fastest startup), one S1 segment, one S2
segment. Segment sizes are kept >= ~256 columns so the PE matmul stream
hides the 128x128 LDWEIGHTS loads.

All device tensors are pre-arranged on host into partition-major
layouts so every DMA is contiguous per partition: activations/weights
for matmul lhsT/rhs always have the contraction dim chunked as
[pi=128, po, free].
"""

import itertools

import numpy as np
import ml_dtypes

import concourse.mybir as mybir
from concourse import bacc
from concourse.tile import TileContext
from concourse import bass_utils

BF16 = mybir.dt.bfloat16
F32 = mybir.dt.float32

D = 2048          # model dim
I = 1408          # expert inter dim
E = 8             # routed experts
TOPK = 2
N_CORES = 8
DPO = D // 128    # 16 chunks of the model dim
IPO = I // 128    # 11 chunks of the inter dim

_BUILD_CACHE = {}


def _c_blocks(C):
    """Split C columns into near-equal blocks <= 512 (PSUM bank limit).
    Near-equal (285/284 rather than 512/57) keeps every matmul well above
    the ~60-cycle small-N floor."""
    nblk = -(-C // 512)
    per = -(-C // nblk)
    blocks = []
    off = 0
    while off < C:
        w = min(per, C - off)
        blocks.append((off, w))
        off += w
    return blocks


def _build(sizes, TS):
    """Per-core Bass kernel: shared job (TS tokens) + one routed job per
    entry in `sizes`. Same NEFF runs SPMD on all 8 cores."""
    nc = bacc.Bacc("TRN2", debug=False, enable_asserts=False,
                   num_devices=N_CORES, enable_partition_id=False)

    def din(name, shape, dt=BF16):
        return nc.dram_tensor(name, shape, dt, kind="ExternalInput").ap()

    def dout(name, shape, dt=BF16):
        return nc.dram_tensor(name, shape, dt, kind="ExternalOutput").ap()

    Silu = mybir.ActivationFunctionType.Silu

    # jobs: (name, C, scaled, paired)
    jobs = [("s", TS, False, True)]
    for j, sz in enumerate(sizes):
        jobs.append((f"r{j}", sz, True, False))

    ins = {}
    for jn, CJ, scaled, _ in jobs:
        ins[jn] = {
            "x": din(f"x_{jn}", [128, DPO, CJ]),
            "w1": din(f"w1_{jn}", [IPO, 128, D]),
            "w3": din(f"w3_{jn}", [IPO, 128, D]),
            "w2": din(f"w2_{jn}", [DPO, 128, I]),
            "y": dout(f"y_{jn}", [128, DPO, CJ]),
        }
        if scaled:
            ins[jn]["cw"] = din(f"cw_{jn}", [128, CJ], F32)
    # packed startup tensors, consumption-ordered:
    #   pk_a = (w1_s[i0]d0 | w3_s[i0]d0 | x_s d0 | w1d1 w3d1 w1d2 w3d2 w1d3 w3d3)
    #   pk_b = same for i1 with x_s d1
    #   pk_c = (w1_s[i2]d0 w3d0 w1d1 w3d1 w1d2 w3d2 w1d3 w3d3)  (no x)
    # DMA'd in chunks so the first matmuls' deps are one small transfer.
    PKN = 8 * 128 + 512
    pk_a = din("pk_a", [128, PKN])
    pk_b = din("pk_b", [128, PKN])
    pk_c = din("pk_c", [128, 1024])

    def _pk_off(wn, d):
        # offsets within pk_a/pk_b for the w chunks
        if d == 0:
            return 0 if wn == "w1" else 128
        base = 256 + 512
        return base + ((d - 1) * 2 + (0 if wn == "w1" else 1)) * 128

    def _pkc_off(wn, d):
        return (2 * d + (0 if wn == "w1" else 1)) * 128

    with TileContext(nc) as tc:
        with tc.tile_pool(name="main", bufs=1) as pool, \
             tc.tile_pool(name="psum", bufs=1, space="PSUM") as pp:

            def w13_tile(jn, i, wn):
                return pool.tile([128, DPO, 128], BF16, tag="w13", bufs=8,
                                 name=f"{wn}_{jn}_{i}")

            def w13_src(jn, wn, i):
                return ins[jn][wn][i].rearrange("p (a b) -> p a b", a=DPO)

            def w2_tile(jn, do):
                return pool.tile([128, IPO, 128], BF16, tag="w2", bufs=5,
                                 name=f"w2_{jn}_{do}")

            x_sb = {}
            H_sb = {}
            cw_sb = {}
            for jn, CJ, scaled, _ in jobs:
                x_sb[jn] = pool.tile([128, DPO, CJ], BF16, tag=f"x_{jn}",
                                     bufs=1, name=f"x_{jn}")
                H_sb[jn] = pool.tile([128, IPO, CJ], BF16, tag=f"H_{jn}",
                                     bufs=1, name=f"H_{jn}")
                if scaled:
                    cw_sb[jn] = pool.tile([128, CJ], F32, tag=f"cw_{jn}",
                                          bufs=1, name=f"cw_{jn}")

            # ---- PE warmup: with the triple-start the early stream has
            # DMA-supply margin, so the HAM cold-clock ramp (~2us) is the
            # exposed startup cost. ~45 dummy matmuls during the pre-DMA
            # dead window cross the ~3.4us HAM warm-up and end before the
            # first real matmul's deps land (~10us), with the idle gap
            # kept under the ~3.4us re-throttle window.
            wu = pool.tile([128, 128], BF16, tag="wu", bufs=1, name="wu")
            nc.vector.memset(wu[:], 0.0)
            wu_ps = pp.tile([128, 512], F32, tag="ps", bufs=8, name="wu_ps")
            NWU = 45
            for k in range(NWU):
                nc.tensor.matmul(wu_ps[:, 0:128], wu[:], wu[:],
                                 start=(k == 0), stop=(k == NWU - 1))

            # ---- startup DMAs, in consumption order at ~DMA rate.
            # The shared job opens with an i-TRIPLE (i0,i1,i2): its d-loop
            # consumes at 1.28us/slice so the ~5.3MB of startup data keeps
            # ahead of the PE with margin (an i-pair start is supply-bound
            # and stalls ~5us). Packs carry d0..3 of the triple's weights
            # plus x d0/d1; remainders stream as 4-d chunks interleaved
            # with x slices in consumption order.
            pk_a_sb = pool.tile([128, PKN], BF16, tag="pk",
                                bufs=3, name="pk_a")
            pk_b_sb = pool.tile([128, PKN], BF16, tag="pk",
                                bufs=3, name="pk_b")
            pk_c_sb = pool.tile([128, 1024], BF16, tag="pkc",
                                bufs=1, name="pk_c")
            # first chunks = exactly the d0/d1 deps of the first matmuls
            nc.sync.dma_start(pk_a_sb[:, 0:768], pk_a[:, 0:768])
            nc.sync.dma_start(pk_b_sb[:, 0:768], pk_b[:, 0:768])
            nc.sync.dma_start(pk_c_sb[:, 0:512], pk_c[:, 0:512])
            nc.sync.dma_start(pk_a_sb[:, 768:], pk_a[:, 768:])
            nc.sync.dma_start(pk_b_sb[:, 768:], pk_b[:, 768:])
            nc.sync.dma_start(pk_c_sb[:, 512:], pk_c[:, 512:])
            pre_w = {}
            for i in (0, 1, 2):
                for wn in ("w1", "w3"):
                    pre_w[("s", i, wn)] = w13_tile("s", i, wn)
            xs = x_sb["s"]
            xd = ins["s"]["x"]
            nc.sync.dma_start(xs[:, 2:4, :], xd[:, 2:4, :])
            nc.sync.dma_start(xs[:, 4:8, :], xd[:, 4:8, :])
            for dlo in (4, 8, 12):
                if dlo > 4:
                    nc.sync.dma_start(xs[:, dlo:dlo + 4, :],
                                      xd[:, dlo:dlo + 4, :])
                for i in (0, 1, 2):
                    for wn in ("w1", "w3"):
                        nc.sync.dma_start(
                            pre_w[("s", i, wn)][:, dlo:dlo + 4, :],
                            w13_src("s", wn, i)[:, dlo:dlo + 4, :])

            def lhsT_ap(jn, i, wn, d, wts):
                if jn == "s" and i in (0, 1, 2) and d < 4:
                    if i == 2:
                        return pk_c_sb[:, _pkc_off(wn, d):
                                       _pkc_off(wn, d) + 128]
                    pk = pk_a_sb if i == 0 else pk_b_sb
                    o = _pk_off(wn, d)
                    return pk[:, o: o + 128]
                return wts[(i, wn)][:, d, :]

            def rhs_ap(jn, d, off, w):
                if jn == "s" and d < 2:
                    pk = pk_a_sb if d == 0 else pk_b_sb
                    return pk[:, 256 + off: 256 + off + w]
                return x_sb[jn][:, d, off:off + w]

            def phase_a(jn, CJ, scaled, groups, pre=None):
                cbs = _c_blocks(CJ)
                H = H_sb[jn]
                for gidx, ii in enumerate(groups):
                    wts = {}
                    for i in ii:
                        for wn in ("w1", "w3"):
                            if pre is not None and (jn, i, wn) in pre:
                                wts[(i, wn)] = pre[(jn, i, wn)]
                            else:
                                t = w13_tile(jn, i, wn)
                                nc.sync.dma_start(t[:], w13_src(jn, wn, i))
                                wts[(i, wn)] = t
                    if jn == "s" and gidx in (2, 3):
                        # shared stream is rolling and the startup DMA
                        # backlog has drained: enqueue one routed job's
                        # x and cw (needed ~100us later) behind this
                        # group's weights; split across two groups so
                        # shared weight prefetch never falls behind.
                        jidx2 = gidx - 1
                        if jidx2 < len(jobs):
                            jn2 = jobs[jidx2][0]
                            nc.sync.dma_start(x_sb[jn2][:],
                                              ins[jn2]["x"][:])
                            nc.sync.dma_start(cw_sb[jn2][:],
                                              ins[jn2]["cw"][:])
                    ps = {}
                    for i in ii:
                        for op in (1, 3):
                            for bi, (off, w) in enumerate(cbs):
                                ps[(i, op, bi)] = pp.tile(
                                    [128, w], F32, tag="ps", bufs=8,
                                    name=f"p{op}_{jn}_{i}_{bi}")
                    for d in range(DPO):
                        for i in ii:
                            for op in (1, 3):
                                wn = "w1" if op == 1 else "w3"
                                for bi, (off, w) in enumerate(cbs):
                                    nc.tensor.matmul(
                                        ps[(i, op, bi)][:],
                                        lhsT_ap(jn, i, wn, d, wts),
                                        rhs_ap(jn, d, off, w),
                                        start=(d == 0), stop=(d == DPO - 1))
                    for i in ii:
                        for bi, (off, w) in enumerate(cbs):
                            s_t = pool.tile([128, w], F32, tag="act1",
                                            bufs=6, name=f"s_{jn}_{i}_{bi}")
                            nc.scalar.activation(s_t[:], ps[(i, 1, bi)][:],
                                                 Silu)
                            if scaled:
                                t_t = pool.tile([128, w], F32, tag="act2",
                                                bufs=6,
                                                name=f"t_{jn}_{i}_{bi}")
                                nc.vector.tensor_mul(
                                    t_t[:], ps[(i, 3, bi)][:],
                                    cw_sb[jn][:, off:off + w])
                                nc.vector.tensor_mul(H[:, i, off:off + w],
                                                     s_t[:], t_t[:])
                            else:
                                nc.vector.tensor_mul(H[:, i, off:off + w],
                                                     s_t[:],
                                                     ps[(i, 3, bi)][:])

            def phase_b(jn, CJ, pre_w2=None):
                cbs = _c_blocks(CJ)
                H = H_sb[jn]
                for do in range(DPO):
                    if pre_w2 is not None and do == 0:
                        w2_sb = pre_w2
                    else:
                        w2_sb = w2_tile(jn, do)
                        nc.sync.dma_start(
                            w2_sb[:],
                            ins[jn]["w2"][do].rearrange("p (a b) -> p a b",
                                                        a=IPO))
                    pys = []
                    for bi, (off, w) in enumerate(cbs):
                        pys.append(pp.tile([128, w], F32, tag="ps", bufs=8,
                                           name=f"py_{jn}_{do}_{bi}"))
                    for i in range(IPO):
                        for bi, (off, w) in enumerate(cbs):
                            nc.tensor.matmul(
                                pys[bi][:], w2_sb[:, i, :],
                                H[:, i, off:off + w],
                                start=(i == 0), stop=(i == IPO - 1))
                    for bi, (off, w) in enumerate(cbs):
                        y_t = pool.tile([128, w], BF16, tag="yo", bufs=8,
                                        name=f"y_{jn}_{do}_{bi}")
                        nc.vector.tensor_copy(y_t[:], pys[bi][:])
                        nc.sync.dma_start(
                            ins[jn]["y"][:, do, off:off + w], y_t[:])

            # ---- job sequence with cross-job weight prefetch ----
            # shared job: triple start (supply-friendly), then a single so
            # its 2 PSUM tiles only wait on i0's DVE drain, then pairs.
            s_groups = [[0, 1, 2], [3], [4, 5], [6, 7], [8, 9], [10]]
            njobs = len(jobs)
            for jidx, (jn, CJ, scaled, paired) in enumerate(jobs):
                groups = s_groups if paired else [[i] for i in range(IPO)]
                pre = pre_w if jidx == 0 else pre_next
                phase_a(jn, CJ, scaled, groups, pre=pre)
                # prefetch next job's first weight pair before our phase B
                pre_next = {}
                if jidx + 1 < njobs:
                    jn2 = jobs[jidx + 1][0]
                    for i in (0,):
                        for wn in ("w1", "w3"):
                            t = w13_tile(jn2, i, wn)
                            nc.sync.dma_start(t[:], w13_src(jn2, wn, i))
                            pre_next[(jn2, i, wn)] = t
                # prefetch our w2[do=0]
                w2_first = w2_tile(jn, 0)
                nc.sync.dma_start(
                    w2_first[:],
                    ins[jn]["w2"][0].rearrange("p (a b) -> p a b", a=IPO))
                phase_b(jn, CJ, pre_w2=w2_first)

    nc.finalize()
    return nc


def _get_kernel(sizes, TS):
    key = (tuple(sizes), TS)
    if key not in _BUILD_CACHE:
        _BUILD_CACHE[key] = _build(tuple(sizes), TS)
    return _BUILD_CACHE[key]


# ---------------- host-side planning ----------------

def _plan_sizes(counts):
    """Choose (s1, s2) segment sizes and per-expert allocation
    (k1_e, k2_e) minimizing modeled PE stream time, with every segment
    >= 256 columns so matmul streaming hides LDWEIGHTS."""
    counts = list(counts)
    ne = len(counts)
    LDW = 107.0

    def chunk_ns(C):
        if C <= 0:
            return 0.0
        nblk = -(-C // 512)
        return max(LDW, C / 2.4 + 2.5 * nblk)

    def feas_s2(resid, s2):
        return sum(-(-r // s2) for r in resid if r > 0) <= ne

    cands = sorted({-(-n // j) for n in counts for j in (1, 2, 3)} |
                   {max(counts)})
    best = None
    for s1 in cands:
        if s1 < 256:
            continue
        caps = [min(3, -(-n // s1)) for n in counts]
        for k1 in itertools.product(*[range(c + 1) for c in caps]):
            if sum(k1) > ne:
                continue
            resid = [max(0, n - k * s1) for n, k in zip(counts, k1)]
            if all(r == 0 for r in resid):
                # second class would be empty but still cost PE time;
                # the 1-class candidate below covers this case
                continue
            lo, hi = 256, max(counts)
            if not feas_s2(resid, hi):
                continue
            while lo < hi:
                mid = (lo + hi) // 2
                if feas_s2(resid, mid):
                    hi = mid
                else:
                    lo = mid + 1
            t = 528 * (chunk_ns(s1) + chunk_ns(lo))
            if best is None or t < best[0]:
                k2 = [-(-r // lo) if r > 0 else 0 for r in resid]
                best = (t, (s1, lo), list(k1), k2)
    # 1-class fallback: every expert one segment of max(counts)
    t1 = 528 * chunk_ns(max(counts))
    if best is None or t1 < best[0]:
        best = (t1, (max(counts),), [1] * ne, [0] * ne)
    _, sizes, k1, k2 = best
    return sizes, k1, k2


def _pm(a, po):
    """[N, po*128] -> partition-major [128, po, N] contiguous."""
    n = a.shape[0]
    return np.ascontiguousarray(
        a.T.reshape(po, 128, n).transpose(1, 0, 2))


def _wA_layout(wm):  # [I, D] -> [IPO, 128, D]; [ib,pi,po*128+ic]
    return np.ascontiguousarray(
        wm.T.reshape(DPO, 128, IPO, 128).transpose(2, 1, 0, 3)
    ).reshape(IPO, 128, D)


def _wB_layout(wm):  # [D, I] -> [DPO, 128, I]; [db,pi,po*128+dc]
    return np.ascontiguousarray(
        wm.T.reshape(IPO, 128, DPO, 128).transpose(2, 1, 0, 3)
    ).reshape(DPO, 128, I)


def kernel(x, gate_w, gate_b, w1, w2, w3, sw1, sw2, sw3):
    bf16 = ml_dtypes.bfloat16
    x = np.asarray(x)
    gate_w = np.asarray(gate_w, dtype=np.float32)
    gate_b = np.asarray(gate_b, dtype=np.float32)
    w1 = np.asarray(w1)
    w2 = np.asarray(w2)
    w3 = np.asarray(w3)
    sw1 = np.asarray(sw1)
    sw2 = np.asarray(sw2)
    sw3 = np.asarray(sw3)

    B, S, Dx = x.shape
    assert Dx == D
    T = B * S
    TS = T // N_CORES
    xt = x.reshape(T, D)

    # ---- gate (fp32, mirrors reference: sqrt(softplus), top-2 on biased) ----
    xf = xt.astype(np.float32)
    logits = xf @ gate_w.T
    scores = np.sqrt(np.log1p(np.exp(-np.abs(logits)))
                     + np.maximum(logits, 0.0))
    biased = scores + gate_b
    idx = np.argsort(-biased, axis=1, kind="stable")[:, :TOPK]
    cw = np.zeros((T, E), dtype=np.float32)
    np.put_along_axis(cw, idx, np.take_along_axis(scores, idx, axis=1), axis=1)

    sel = np.zeros((T, E), dtype=bool)
    np.put_along_axis(sel, idx, True, axis=1)
    tok_lists = [np.nonzero(sel[:, e])[0] for e in range(E)]
    counts = [len(t) for t in tok_lists]

    sizes, k1, k2 = _plan_sizes(counts)

    # build per-class piece lists: (expert, token_idx_array)
    nclass = len(sizes)
    pieces = [[] for _ in range(nclass)]
    for e in range(E):
        toks = tok_lists[e]
        pos = 0
        alloc = [(0, k1[e])] + ([(1, k2[e])] if nclass > 1 else [])
        for cls, k in alloc:
            for _ in range(k):
                if pos >= len(toks):
                    break
                take = min(sizes[cls], len(toks) - pos)
                pieces[cls].append((e, toks[pos:pos + take]))
                pos += take
        assert pos == len(toks), f"expert {e} unplaced tokens"
    for cls in range(nclass):
        assert len(pieces[cls]) <= N_CORES, \
            f"class {cls} needs {len(pieces[cls])} > {N_CORES} segments"
        while len(pieces[cls]) < N_CORES:
            pieces[cls].append((0, np.array([], dtype=np.int64)))

    nc = _get_kernel(sizes, TS)

    # weight layout transforms, cached per expert
    wa_cache, wb_cache = {}, {}

    def get_w(e):
        if e not in wa_cache:
            wa_cache[e] = (_wA_layout(w1[e]), _wA_layout(w3[e]))
            wb_cache[e] = _wB_layout(w2[e])
        return wa_cache[e][0], wa_cache[e][1], wb_cache[e]

    sw1t = _wA_layout(sw1)
    sw3t = _wA_layout(sw3)
    sw2t = _wB_layout(sw2)

    in_maps = []
    for c in range(N_CORES):
        xs_pm = _pm(xt[c * TS:(c + 1) * TS], DPO)
        # packed startup, consumption-ordered:
        # (w1[i]d0 | w3[i]d0 | x_s d_i | w1d1 w3d1 w1d2 w3d2 w1d3 w3d3)
        def mk_pk(i, xsl):
            parts = [sw1t[i][:, 0:128], sw3t[i][:, 0:128], xsl]
            for dd in range(1, 4):
                parts.append(sw1t[i][:, dd * 128:(dd + 1) * 128])
                parts.append(sw3t[i][:, dd * 128:(dd + 1) * 128])
            return np.ascontiguousarray(np.concatenate(parts, axis=1))
        pk_a = mk_pk(0, xs_pm[:, 0, :])
        pk_b = mk_pk(1, xs_pm[:, 1, :])
        # pk_c: i2 weights only, (w1d w3d) interleaved per d
        pk_c = np.ascontiguousarray(np.concatenate(
            [w[:, dd * 128:(dd + 1) * 128]
             for dd in range(4) for w in (sw1t[2], sw3t[2])], axis=1))
        m = {
            "x_s": xs_pm,
            "w1_s": sw1t, "w3_s": sw3t, "w2_s": sw2t,
            "pk_a": pk_a, "pk_b": pk_b, "pk_c": pk_c,
        }
        for cls in range(nclass):
            e, toks = pieces[cls][c]
            CJ = sizes[cls]
            xg = np.zeros((CJ, D), dtype=bf16)
            cwe = np.zeros((CJ,), dtype=np.float32)
            cnt = len(toks)
            if cnt:
                xg[:cnt] = xt[toks]
                cwe[:cnt] = cw[toks, e]
            w1t, w3t, w2t = get_w(e)
            jn = f"r{cls}"
            m[f"x_{jn}"] = _pm(xg, DPO)
            m[f"cw_{jn}"] = np.ascontiguousarray(
                np.broadcast_to(cwe[None, :], (128, CJ)))
            m[f"w1_{jn}"] = w1t
            m[f"w3_{jn}"] = w3t
            m[f"w2_{jn}"] = w2t
        in_maps.append(m)

    res = bass_utils.run_bass_kernel_spmd(
        nc, in_maps, core_ids=list(range(N_CORES)))
    global LAST_RESULT
    LAST_RESULT = res

    # ---- unshard + combine (bf16, reference expert order) ----
    y = np.zeros((T, D), dtype=bf16)
    for e in range(E):
        for cls in range(nclass):
            for c in range(N_CORES):
                pe, toks = pieces[cls][c]
                if pe != e or len(toks) == 0:
                    continue
                CJ = sizes[cls]
                ye = res.results[c][f"y_r{cls}"]          # [128, DPO, CJ]
                ye_tok = ye.transpose(2, 1, 0).reshape(CJ, D)
                y[toks] = y[toks] + ye_tok[:len(toks)]
    z = np.concatenate(
        [res.results[c]["y_s"].transpose(2, 1, 0).reshape(TS, D)
         for c in range(N_CORES)], axis=0)
    out = (y + z).reshape(B, S, D)
    return out.astype(x.dtype)


# revision 27
# speedup vs baseline: 1.0008x; 1.0008x over previous
"""MoE (8 routed experts, top-2, + shared expert) on 8 TRN2 NeuronCores.

Strategy: expert-parallel with load-balanced segmentation. Host computes
the gate (fp32 numpy, mirroring the reference), then packs the 8192
routed (token, expert) pairs into 16 expert-pure segments — 8 of size S1
and 8 of size S2 (sizes chosen per the actual expert counts so
S1+S2 ~= 1058 vs the 1152 max-count padding of naive expert-parallel).
Each core runs three SwiGLU jobs: a 512-token slice of the shared
expert (first: smallest x, fastest startup), one S1 segment, one S2
segment. Segment sizes are kept >= ~256 columns so the PE matmul stream
hides the 128x128 LDWEIGHTS loads.

All device tensors are pre-arranged on host into partition-major
layouts so every DMA is contiguous per partition: activations/weights
for matmul lhsT/rhs always have the contraction dim chunked as
[pi=128, po, free].
"""

import itertools

import numpy as np
import ml_dtypes

import concourse.mybir as mybir
from concourse import bacc
from concourse.tile import TileContext
from concourse import bass_utils

BF16 = mybir.dt.bfloat16
F32 = mybir.dt.float32

D = 2048          # model dim
I = 1408          # expert inter dim
E = 8             # routed experts
TOPK = 2
N_CORES = 8
DPO = D // 128    # 16 chunks of the model dim
IPO = I // 128    # 11 chunks of the inter dim

_BUILD_CACHE = {}


def _c_blocks(C):
    """Split C columns into near-equal blocks <= 512 (PSUM bank limit).
    Near-equal (285/284 rather than 512/57) keeps every matmul well above
    the ~60-cycle small-N floor."""
    nblk = -(-C // 512)
    per = -(-C // nblk)
    blocks = []
    off = 0
    while off < C:
        w = min(per, C - off)
        blocks.append((off, w))
        off += w
    return blocks


def _build(sizes, TS):
    """Per-core Bass kernel: shared job (TS tokens) + one routed job per
    entry in `sizes`. Same NEFF runs SPMD on all 8 cores."""
    nc = bacc.Bacc("TRN2", debug=False, enable_asserts=False,
                   num_devices=N_CORES, enable_partition_id=False)

    def din(name, shape, dt=BF16):
        return nc.dram_tensor(name, shape, dt, kind="ExternalInput").ap()

    def dout(name, shape, dt=BF16):
        return nc.dram_tensor(name, shape, dt, kind="ExternalOutput").ap()

    Silu = mybir.ActivationFunctionType.Silu

    # jobs: (name, C, scaled, paired)
    jobs = [("s", TS, False, True)]
    for j, sz in enumerate(sizes):
        jobs.append((f"r{j}", sz, True, False))

    ins = {}
    for jn, CJ, scaled, _ in jobs:
        ins[jn] = {
            "x": din(f"x_{jn}", [128, DPO, CJ]),
            "w1": din(f"w1_{jn}", [IPO, 128, D]),
            "w3": din(f"w3_{jn}", [IPO, 128, D]),
            "w2": din(f"w2_{jn}", [DPO, 128, I]),
            "y": dout(f"y_{jn}", [128, DPO, CJ]),
        }
        if scaled:
            ins[jn]["cw"] = din(f"cw_{jn}", [128, CJ], F32)
    # packed startup tensors, consumption-ordered:
    #   pk_a = (w1_s[i0]d0 | w3_s[i0]d0 | x_s d0 | w1d1 w3d1 w1d2 w3d2 w1d3 w3d3)
    #   pk_b = same for i1 with x_s d1
    #   pk_c = (w1_s[i2]d0 w3d0 w1d1 w3d1 w1d2 w3d2 w1d3 w3d3)  (no x)
    # DMA'd in chunks so the first matmuls' deps are one small transfer.
    PKN = 8 * 128 + 512
    pk_a = din("pk_a", [128, PKN])
    pk_b = din("pk_b", [128, PKN])
    pk_c = din("pk_c", [128, 1024])

    def _pk_off(wn, d):
        # offsets within pk_a/pk_b for the w chunks
        if d == 0:
            return 0 if wn == "w1" else 128
        base = 256 + 512
        return base + ((d - 1) * 2 + (0 if wn == "w1" else 1)) * 128

    def _pkc_off(wn, d):
        return (2 * d + (0 if wn == "w1" else 1)) * 128

    with TileContext(nc) as tc:
        with tc.tile_pool(name="main", bufs=1) as pool, \
             tc.tile_pool(name="psum", bufs=1, space="PSUM") as pp:

            def w13_tile(jn, i, wn):
                return pool.tile([128, DPO, 128], BF16, tag="w13", bufs=8,
                                 name=f"{wn}_{jn}_{i}")

            def w13_src(jn, wn, i):
                return ins[jn][wn][i].rearrange("p (a b) -> p a b", a=DPO)

            def w2_tile(jn, do):
                return pool.tile([128, IPO, 128], BF16, tag="w2", bufs=5,
                                 name=f"w2_{jn}_{do}")

            x_sb = {}
            H_sb = {}
            cw_sb = {}
            for jn, CJ, scaled, _ in jobs:
                x_sb[jn] = pool.tile([128, DPO, CJ], BF16, tag=f"x_{jn}",
                                     bufs=1, name=f"x_{jn}")
                H_sb[jn] = pool.tile([128, IPO, CJ], BF16, tag=f"H_{jn}",
                                     bufs=1, name=f"H_{jn}")
                if scaled:
                    cw_sb[jn] = pool.tile([128, CJ], F32, tag=f"cw_{jn}",
                                          bufs=1, name=f"cw_{jn}")

            # ---- startup DMAs, in consumption order at ~DMA rate.
            # The shared job opens with an i-TRIPLE (i0,i1,i2): its d-loop
            # consumes at 1.28us/slice so the ~5.3MB of startup data keeps
            # ahead of the PE with margin (an i-pair start is supply-bound
            # and stalls ~5us). Packs carry d0..3 of the triple's weights
            # plus x d0/d1; remainders stream as 4-d chunks interleaved
            # with x slices in consumption order.
            pk_a_sb = pool.tile([128, PKN], BF16, tag="pk",
                                bufs=3, name="pk_a")
            pk_b_sb = pool.tile([128, PKN], BF16, tag="pk",
                                bufs=3, name="pk_b")
            pk_c_sb = pool.tile([128, 1024], BF16, tag="pkc",
                                bufs=1, name="pk_c")
            # first chunks = exactly the d0/d1 deps of the first matmuls
            nc.sync.dma_start(pk_a_sb[:, 0:768], pk_a[:, 0:768])
            nc.sync.dma_start(pk_b_sb[:, 0:768], pk_b[:, 0:768])
            nc.sync.dma_start(pk_c_sb[:, 0:512], pk_c[:, 0:512])
            nc.sync.dma_start(pk_a_sb[:, 768:], pk_a[:, 768:])
            nc.sync.dma_start(pk_b_sb[:, 768:], pk_b[:, 768:])
            nc.sync.dma_start(pk_c_sb[:, 512:], pk_c[:, 512:])
            pre_w = {}
            for i in (0, 1, 2):
                for wn in ("w1", "w3"):
                    pre_w[("s", i, wn)] = w13_tile("s", i, wn)
            xs = x_sb["s"]
            xd = ins["s"]["x"]
            nc.sync.dma_start(xs[:, 2:4, :], xd[:, 2:4, :])
            nc.sync.dma_start(xs[:, 4:8, :], xd[:, 4:8, :])
            for dlo in (4, 8, 12):
                if dlo > 4:
                    nc.sync.dma_start(xs[:, dlo:dlo + 4, :],
                                      xd[:, dlo:dlo + 4, :])
                for i in (0, 1, 2):
                    for wn in ("w1", "w3"):
                        nc.sync.dma_start(
                            pre_w[("s", i, wn)][:, dlo:dlo + 4, :],
                            w13_src("s", wn, i)[:, dlo:dlo + 4, :])

            def lhsT_ap(jn, i, wn, d, wts):
                if jn == "s" and i in (0, 1, 2) and d < 4:
                    if i == 2:
                        return pk_c_sb[:, _pkc_off(wn, d):
                                       _pkc_off(wn, d) + 128]
                    pk = pk_a_sb if i == 0 else pk_b_sb
                    o = _pk_off(wn, d)
                    return pk[:, o: o + 128]
                return wts[(i, wn)][:, d, :]

            def rhs_ap(jn, d, off, w):
                if jn == "s" and d < 2:
                    pk = pk_a_sb if d == 0 else pk_b_sb
                    return pk[:, 256 + off: 256 + off + w]
                return x_sb[jn][:, d, off:off + w]

            def phase_a(jn, CJ, scaled, groups, pre=None):
                cbs = _c_blocks(CJ)
                H = H_sb[jn]
                for gidx, ii in enumerate(groups):
                    wts = {}
                    for i in ii:
                        for wn in ("w1", "w3"):
                            if pre is not None and (jn, i, wn) in pre:
                                wts[(i, wn)] = pre[(jn, i, wn)]
                            else:
                                t = w13_tile(jn, i, wn)
                                nc.sync.dma_start(t[:], w13_src(jn, wn, i))
                                wts[(i, wn)] = t
                    if jn == "s" and gidx in (2, 3):
                        # shared stream is rolling and the startup DMA
                        # backlog has drained: enqueue one routed job's
                        # x and cw (needed ~100us later) behind this
                        # group's weights; split across two groups so
                        # shared weight prefetch never falls behind.
                        jidx2 = gidx - 1
                        if jidx2 < len(jobs):
                            jn2 = jobs[jidx2][0]
                            nc.sync.dma_start(x_sb[jn2][:],
                                              ins[jn2]["x"][:])
                            nc.sync.dma_start(cw_sb[jn2][:],
                                              ins[jn2]["cw"][:])
                    ps = {}
                    for i in ii:
                        for op in (1, 3):
                            for bi, (off, w) in enumerate(cbs):
                                ps[(i, op, bi)] = pp.tile(
                                    [128, w], F32, tag="ps", bufs=8,
                                    name=f"p{op}_{jn}_{i}_{bi}")
                    for d in range(DPO):
                        for i in ii:
                            for op in (1, 3):
                                wn = "w1" if op == 1 else "w3"
                                for bi, (off, w) in enumerate(cbs):
                                    nc.tensor.matmul(
                                        ps[(i, op, bi)][:],
                                        lhsT_ap(jn, i, wn, d, wts),
                                        rhs_ap(jn, d, off, w),
                                        start=(d == 0), stop=(d == DPO - 1))
                    for i in ii:
                        for bi, (off, w) in enumerate(cbs):
                            s_t = pool.tile([128, w], F32, tag="act1",
                                            bufs=6, name=f"s_{jn}_{i}_{bi}")
                            nc.scalar.activation(s_t[:], ps[(i, 1, bi)][:],
                                                 Silu)
                            if scaled:
                                t_t = pool.tile([128, w], F32, tag="act2",
                                                bufs=6,
                                                name=f"t_{jn}_{i}_{bi}")
                                nc.vector.tensor_mul(
                                    t_t[:], ps[(i, 3, bi)][:],
                                    cw_sb[jn][:, off:off + w])
                                nc.vector.tensor_mul(H[:, i, off:off + w],
                                                     s_t[:], t_t[:])
                            else:
                                nc.vector.tensor_mul(H[:, i, off:off + w],
                                                     s_t[:],
                                                     ps[(i, 3, bi)][:])

            def phase_b(jn, CJ, pre_w2=None):
                cbs = _c_blocks(CJ)
                H = H_sb[jn]
                for do in range(DPO):
                    if pre_w2 is not None and do == 0:
                        w2_sb = pre_w2
                    else:
                        w2_sb = w2_tile(jn, do)
                        nc.sync.dma_start(
                            w2_sb[:],
                            ins[jn]["w2"][do].rearrange("p (a b) -> p a b",
                                                        a=IPO))
                    pys = []
                    for bi, (off, w) in enumerate(cbs):
                        pys.append(pp.tile([128, w], F32, tag="ps", bufs=8,
                                           name=f"py_{jn}_{do}_{bi}"))
                    for i in range(IPO):
                        for bi, (off, w) in enumerate(cbs):
                            nc.tensor.matmul(
                                pys[bi][:], w2_sb[:, i, :],
                                H[:, i, off:off + w],
                                start=(i == 0), stop=(i == IPO - 1))
                    for bi, (off, w) in enumerate(cbs):
                        y_t = pool.tile([128, w], BF16, tag="yo", bufs=8,
                                        name=f"y_{jn}_{do}_{bi}")
                        nc.vector.tensor_copy(y_t[:], pys[bi][:])
                        nc.sync.dma_start(
                            ins[jn]["y"][:, do, off:off + w], y_t[:])

            # ---- job sequence with cross-job weight prefetch ----
            # shared job: triple start (supply-friendly), then a single so
            # its 2 PSUM tiles only wait on i0's DVE drain, then pairs.
            s_groups = [[0, 1, 2], [3], [4, 5], [6, 7], [8, 9], [10]]
            njobs = len(jobs)
            for jidx, (jn, CJ, scaled, paired) in enumerate(jobs):
                groups = s_groups if paired else [[i] for i in range(IPO)]
                pre = pre_w if jidx == 0 else pre_next
                phase_a(jn, CJ, scaled, groups, pre=pre)
                # prefetch next job's first weight pair before our phase B
                pre_next = {}
                if jidx + 1 < njobs:
                    jn2 = jobs[jidx + 1][0]
                    for i in (0,):
                        for wn in ("w1", "w3"):
                            t = w13_tile(jn2, i, wn)
                            nc.sync.dma_start(t[:], w13_src(jn2, wn, i))
                            pre_next[(jn2, i, wn)] = t
                # prefetch our w2[do=0]
                w2_first = w2_tile(jn, 0)
                nc.sync.dma_start(
                    w2_first[:],
                    ins[jn]["w2"][0].rearrange("p (a b) -> p a b", a=IPO))
                phase_b(jn, CJ, pre_w2=w2_first)

    nc.finalize()
    return nc


def _get_kernel(sizes, TS):
    key = (tuple(sizes), TS)
    if key not in _BUILD_CACHE:
        _BUILD_CACHE[key] = _build(tuple(sizes), TS)
    return _BUILD_CACHE[key]


# ---------------- host-side planning ----------------

def _plan_sizes(counts):
    """Choose (s1, s2) segment sizes and per-expert allocation
    (k1_e, k2_e) minimizing modeled PE stream time, with every segment
    >= 256 columns so matmul streaming hides LDWEIGHTS."""
    counts = list(counts)
    ne = len(counts)
    LDW = 107.0

    def chunk_ns(C):
        if C <= 0:
            return 0.0
        nblk = -(-C // 512)
        return max(LDW, C / 2.4 + 2.5 * nblk)

    def feas_s2(resid, s2):
        return sum(-(-r // s2) for r in resid if r > 0) <= ne

    cands = sorted({-(-n // j) for n in counts for j in (1, 2, 3)} |
                   {max(counts)})
    best = None
    for s1 in cands:
        if s1 < 256:
            continue
        caps = [min(3, -(-n // s1)) for n in counts]
        for k1 in itertools.product(*[range(c + 1) for c in caps]):
            if sum(k1) > ne:
                continue
            resid = [max(0, n - k * s1) for n, k in zip(counts, k1)]
            if all(r == 0 for r in resid):
                # second class would be empty but still cost PE time;
                # the 1-class candidate below covers this case
                continue
            lo, hi = 256, max(counts)
            if not feas_s2(resid, hi):
                continue
            while lo < hi:
                mid = (lo + hi) // 2
                if feas_s2(resid, mid):
                    hi = mid
                else:
                    lo = mid + 1
            t = 528 * (chunk_ns(s1) + chunk_ns(lo))
            if best is None or t < best[0]:
                k2 = [-(-r // lo) if r > 0 else 0 for r in resid]
                best = (t, (s1, lo), list(k1), k2)
    # 1-class fallback: every expert one segment of max(counts)
    t1 = 528 * chunk_ns(max(counts))
    if best is None or t1 < best[0]:
        best = (t1, (max(counts),), [1] * ne, [0] * ne)
    _, sizes, k1, k2 = best
    return sizes, k1, k2


def _pm(a, po):
    """[N, po*128] -> partition-major [128, po, N] contiguous."""
    n = a.shape[0]
    return np.ascontiguousarray(
        a.T.reshape(po, 128, n).transpose(1, 0, 2))


def _wA_layout(wm):  # [I, D] -> [IPO, 128, D]; [ib,pi,po*128+ic]
    return np.ascontiguousarray(
        wm.T.reshape(DPO, 128, IPO, 128).transpose(2, 1, 0, 3)
    ).reshape(IPO, 128, D)


def _wB_layout(wm):  # [D, I] -> [DPO, 128, I]; [db,pi,po*128+dc]
    return np.ascontiguousarray(
        wm.T.reshape(IPO, 128, DPO, 128).transpose(2, 1, 0, 3)
    ).reshape(DPO, 128, I)


def kernel(x, gate_w, gate_b, w1, w2, w3, sw1, sw2, sw3):
    bf16 = ml_dtypes.bfloat16
    x = np.asarray(x)
    gate_w = np.asarray(gate_w, dtype=np.float32)
    gate_b = np.asarray(gate_b, dtype=np.float32)
    w1 = np.asarray(w1)
    w2 = np.asarray(w2)
    w3 = np.asarray(w3)
    sw1 = np.asarray(sw1)
    sw2 = np.asarray(sw2)
    sw3 = np.asarray(sw3)

    B, S, Dx = x.shape
    assert Dx == D
    T = B * S
    TS = T // N_CORES
    xt = x.reshape(T, D)

    # ---- gate (fp32, mirrors reference: sqrt(softplus), top-2 on biased) ----
    xf = xt.astype(np.float32)
    logits = xf @ gate_w.T
    scores = np.sqrt(np.log1p(np.exp(-np.abs(logits)))
                     + np.maximum(logits, 0.0))
    biased = scores + gate_b
    idx = np.argsort(-biased, axis=1, kind="stable")[:, :TOPK]
    cw = np.zeros((T, E), dtype=np.float32)
    np.put_along_axis(cw, idx, np.take_along_axis(scores, idx, axis=1), axis=1)

    sel = np.zeros((T, E), dtype=bool)
    np.put_along_axis(sel, idx, True, axis=1)
    tok_lists = [np.nonzero(sel[:, e])[0] for e in range(E)]
    counts = [len(t) for t in tok_lists]

    sizes, k1, k2 = _plan_sizes(counts)

    # build per-class piece lists: (expert, token_idx_array)
    nclass = len(sizes)
    pieces = [[] for _ in range(nclass)]
    for e in range(E):
        toks = tok_lists[e]
        pos = 0
        alloc = [(0, k1[e])] + ([(1, k2[e])] if nclass > 1 else [])
        for cls, k in alloc:
            for _ in range(k):
                if pos >= len(toks):
                    break
                take = min(sizes[cls], len(toks) - pos)
                pieces[cls].append((e, toks[pos:pos + take]))
                pos += take
        assert pos == len(toks), f"expert {e} unplaced tokens"
    for cls in range(nclass):
        assert len(pieces[cls]) <= N_CORES, \
            f"class {cls} needs {len(pieces[cls])} > {N_CORES} segments"
        while len(pieces[cls]) < N_CORES:
            pieces[cls].append((0, np.array([], dtype=np.int64)))

    nc = _get_kernel(sizes, TS)

    # weight layout transforms, cached per expert
    wa_cache, wb_cache = {}, {}

    def get_w(e):
        if e not in wa_cache:
            wa_cache[e] = (_wA_layout(w1[e]), _wA_layout(w3[e]))
            wb_cache[e] = _wB_layout(w2[e])
        return wa_cache[e][0], wa_cache[e][1], wb_cache[e]

    sw1t = _wA_layout(sw1)
    sw3t = _wA_layout(sw3)
    sw2t = _wB_layout(sw2)

    in_maps = []
    for c in range(N_CORES):
        xs_pm = _pm(xt[c * TS:(c + 1) * TS], DPO)
        # packed startup, consumption-ordered:
        # (w1[i]d0 | w3[i]d0 | x_s d_i | w1d1 w3d1 w1d2 w3d2 w1d3 w3d3)
        def mk_pk(i, xsl):
            parts = [sw1t[i][:, 0:128], sw3t[i][:, 0:128], xsl]
            for dd in range(1, 4):
                parts.append(sw1t[i][:, dd * 128:(dd + 1) * 128])
                parts.append(sw3t[i][:, dd * 128:(dd + 1) * 128])
            return np.ascontiguousarray(np.concatenate(parts, axis=1))
        pk_a = mk_pk(0, xs_pm[:, 0, :])
        pk_b = mk_pk(1, xs_pm[:, 1, :])
        # pk_c: i2 weights only, (w1d w3d) interleaved per d
        pk_c = np.ascontiguousarray(np.concatenate(
            [w[:, dd * 128:(dd + 1) * 128]
             for dd in range(4) for w in (sw1t[2], sw3t[2])], axis=1))
        m = {
            "x_s": xs_pm,
            "w1_s": sw1t, "w3_s": sw3t, "w2_s": sw2t,
            "pk_a": pk_a, "pk_b": pk_b, "pk_c": pk_c,
        }
        for cls in range(nclass):
            e, toks = pieces[cls][c]
            CJ = sizes[cls]
            xg = np.zeros((CJ, D), dtype=bf16)
            cwe = np.zeros((CJ,), dtype=np.float32)
            cnt = len(toks)
            if cnt:
                xg[:cnt] = xt[toks]
                cwe[:cnt] = cw[toks, e]
            w1t, w3t, w2t = get_w(e)
            jn = f"r{cls}"
            m[f"x_{jn}"] = _pm(xg, DPO)
            m[f"cw_{jn}"] = np.ascontiguousarray(
                np.broadcast_to(cwe[None, :], (128, CJ)))
            m[f"w1_{jn}"] = w1t
            m[f"w3_{jn}"] = w3t
            m[f"w2_{jn}"] = w2t
        in_maps.append(m)

    res = bass_utils.run_bass_kernel_spmd(
        nc, in_maps, core_ids=list(range(N_CORES)))
    global LAST_RESULT
    LAST_RESULT = res

    # ---- unshard + combine (bf16, reference expert order) ----
    y = np.zeros((T, D), dtype=bf16)
    for e in range(E):
        for cls in range(nclass):
            for c in range(N_CORES):
                pe, toks = pieces[cls][c]
                if pe != e or len(toks) == 0:
                    continue
                CJ = sizes[cls]
                ye = res.results[c][f"y_r{cls}"]          # [128, DPO, CJ]
                ye_tok = ye.transpose(2, 1, 0).reshape(CJ, D)
                y[toks] = y[toks] + ye_tok[:len(toks)]
    z = np.concatenate(
        [res.results[c]["y_s"].transpose(2, 1, 0).reshape(TS, D)
         for c in range(N_CORES)], axis=0)
    out = (y + z).reshape(B, S, D)
    return out.astype(x.dtype)
